# revision 20
# baseline (speedup 1.0000x reference)
"""AttentionContext kernel for Trainium2, data-parallel over batch on 8 cores.

Reference computation (B=64, T=2048, D=512 everywhere):
    phi_s = s @ phi_w.T + phi_b                  # [B, D]
    psi_h = einsum('bth,ah->bta', h, psi_w) + psi_b
    e     = einsum('ba,bta->bt', phi_s, psi_h)   # [B, T]
    alpha = softmax(e, axis=-1)
    c     = alpha * h.sum(-1)                    # [B, T]

Algebraic restructuring used here:
    e[b,t] = phi_s[b] . (psi_w @ h[b,t]) + phi_s[b] . psi_b
           = (phi_s[b] @ psi_w) . h[b,t] + const(b)
    The const(b) term is uniform over t, so softmax ignores it -> dropped.
    w = phi_s @ psi_w = s @ (phi_w.T @ psi_w) + phi_b @ psi_w
    Both weight-matrix contractions run over the leading (partition-natural)
    dim, so no weight transposes are needed.

Per core (8 batches): one streaming pass over h computing, per [128t, 512d]
tile, e via a fused DVE scalar_tensor_tensor (multiply + free-dim sum) against
a broadcast w[b], and hsum via ScalarE activation-accumulate (2 of every 16
tiles on DVE tensor_reduce for load balance). Softmax runs on [128, 16]
per-batch blocks with PE matmuls/transposes for cross-partition max/sum.
"""

import numpy as np

import concourse.bass as bass
import concourse.bacc as bacc
import concourse.tile as tile
from concourse import mybir
from concourse import bass_utils
from concourse.masks import make_identity

FP = mybir.dt.float32
ALU = mybir.AluOpType
AF = mybir.ActivationFunctionType

N_CORES = 8
B_LOC = 8          # batches per core
T = 2048
D = 512
P = 128
KC = D // P        # 4 contraction chunks of 128
TI = T // P        # 16 t-tiles per batch
SUP = 8            # t-tiles per DMA super-tile
NSUP = TI // SUP   # 2 super-tiles per batch
# t-tile indices whose hsum runs on DVE instead of ScalarE (load balance;
# walrus rejects accumulating tensor_scalar on the Pool engine, so GpSimd
# cannot help with the free-dim reductions)
DVE_HSUM_TI = {7, 15}


def _emit(nc, tc, variant="full"):
    s = nc.dram_tensor("s", [B_LOC, D], FP, kind="ExternalInput").ap()
    h = nc.dram_tensor("h", [B_LOC, T, D], FP, kind="ExternalInput").ap()
    phi_w = nc.dram_tensor("phi_w", [D, D], FP, kind="ExternalInput").ap()
    phi_b = nc.dram_tensor("phi_b", [D], FP, kind="ExternalInput").ap()
    psi_w = nc.dram_tensor("psi_w", [D, D], FP, kind="ExternalInput").ap()
    w_scr = nc.dram_tensor("w_scr", [B_LOC, D], FP, kind="Internal").ap()
    if variant == "s0":
        c_out = nc.dram_tensor("c", [B_LOC, D], FP, kind="ExternalOutput").ap()
    elif variant == "s1":
        c_out = nc.dram_tensor("c", [P, 2 * P], FP, kind="ExternalOutput").ap()
    else:
        c_out = nc.dram_tensor("c", [B_LOC, T], FP, kind="ExternalOutput").ap()

    with tc.tile_pool(name="consts", bufs=1) as consts:
        # ---------------- stage 0: combined weights ----------------
        phi_w_sb = consts.tile([P, KC, D], FP)   # [a % 128, a // 128, k]
        nc.sync.dma_start(
            out=phi_w_sb, in_=phi_w.rearrange("(ac p) k -> p ac k", p=P)
        )
        psi_w_sb = consts.tile([P, KC, D], FP)   # [a % 128, a // 128, m]
        nc.sync.dma_start(
            out=psi_w_sb, in_=psi_w.rearrange("(ac p) m -> p ac m", p=P)
        )
        phi_b_sb = consts.tile([P, KC], FP)      # [a % 128, a // 128]
        nc.sync.dma_start(out=phi_b_sb, in_=phi_b.rearrange("(ac p) -> p ac", p=P))
        s_sb = consts.tile([B_LOC, D], FP)
        nc.sync.dma_start(out=s_sb, in_=s)

        ident = consts.tile([P, P], FP)
        make_identity(nc, ident)
        ones_1x8 = consts.tile([1, B_LOC], FP)
        nc.vector.memset(ones_1x8, 1.0)
        ones_1x128 = consts.tile([1, P], FP)
        nc.vector.memset(ones_1x128, 1.0)
        neg_1x128 = consts.tile([1, P], FP)
        nc.vector.memset(neg_1x128, -1.0)
        ones_128x1 = consts.tile([P, 1], FP)
        nc.vector.memset(ones_128x1, 1.0)

        # Warm the ACT exp table set early so the ~2.7us load overlaps.
        tiny = consts.tile([1, 1], FP)
        nc.vector.memset(tiny, 0.0)
        nc.scalar.activation(out=tiny, in_=tiny, func=AF.Exp)

        mc_sb = consts.tile([P, KC, D], FP)      # M_c[k, m], k = kc*128 + p
        v_sb = consts.tile([1, D], FP)           # v[m] = phi_b @ psi_w
        sT_sb = consts.tile([P, KC, B_LOC], FP)  # s.T[k, b]
        w_sb = consts.tile([B_LOC, D], FP)       # w[b, m]
        e_all = consts.tile([P, P], FP)          # e[t%128, b*16 + ti]
        hs_all = consts.tile([P, P], FP)         # hsum, same layout
        exp_all = consts.tile([P, P], FP)        # exp(e - max_b), same layout

        with tc.tile_pool(name="psum0", bufs=2, space="PSUM") as psum0:
            # Short dummy-matmul burst overlapping the weight DMAs so HAM
            # un-throttles the PE before the stage-0 matmul chain.
            warm = consts.tile([P, D], FP)
            nc.vector.memset(warm, 0.25)
            warm_ps = psum0.tile([P, D], FP, tag="mc_ps")
            for _ in range(4):
                nc.tensor.matmul(warm_ps, lhsT=ident, rhs=warm)

            # M_c[k, m] = sum_a phi_w[a, k] * psi_w[a, m]
            for kc in range(KC):
                mc_ps = psum0.tile([P, D], FP, tag="mc_ps")
                for ac in range(KC):
                    nc.tensor.matmul(
                        mc_ps,
                        lhsT=phi_w_sb[:, ac, kc * P : (kc + 1) * P],
                        rhs=psi_w_sb[:, ac, :],
                        start=(ac == 0),
                        stop=(ac == KC - 1),
                    )
                nc.vector.tensor_copy(out=mc_sb[:, kc, :], in_=mc_ps)

            # v[m] = sum_a phi_b[a] * psi_w[a, m]
            v_ps = psum0.tile([1, D], FP, tag="v_ps")
            for ac in range(KC):
                nc.tensor.matmul(
                    v_ps,
                    lhsT=phi_b_sb[:, ac : ac + 1],
                    rhs=psi_w_sb[:, ac, :],
                    start=(ac == 0),
                    stop=(ac == KC - 1),
                )
            nc.vector.tensor_copy(out=v_sb, in_=v_ps)

            # s.T chunks via PE transpose
            for kc in range(KC):
                st_ps = psum0.tile([P, B_LOC], FP, tag="st_ps")
                nc.tensor.transpose(
                    st_ps,
                    in_=s_sb[:, kc * P : (kc + 1) * P],
                    identity=ident[:B_LOC, :B_LOC],
                )
                nc.vector.tensor_copy(out=sT_sb[:, kc, :], in_=st_ps)

            # w[b, m] = sum_k sT[k, b] * M_c[k, m] + 1 * v[m]
            w_ps = psum0.tile([B_LOC, D], FP, tag="w_ps")
            for kc in range(KC):
                nc.tensor.matmul(
                    w_ps,
                    lhsT=sT_sb[:, kc, :],
                    rhs=mc_sb[:, kc, :],
                    start=(kc == 0),
                    stop=False,
                )
            nc.tensor.matmul(w_ps, lhsT=ones_1x8, rhs=v_sb, start=False, stop=True)
            nc.vector.tensor_copy(out=w_sb, in_=w_ps)

        if variant == "s0":
            nc.sync.dma_start(out=c_out, in_=w_sb)
            return

        # ---------------- stages 1+2: stream h ----------------
        with (
            tc.tile_pool(name="hpool", bufs=7) as hpool,
            tc.tile_pool(name="wb", bufs=2) as wbpool,
            tc.tile_pool(name="junk", bufs=3) as junk,
            tc.tile_pool(name="small", bufs=2) as small,
            tc.tile_pool(name="psum2", bufs=1, space="PSUM") as psum2,
        ):
            # stage w to DRAM once; per-b partition-broadcast loads from DRAM
            nc.sync.dma_start(out=w_scr, in_=w_sb)
            for b in range(B_LOC):
                w_bc = wbpool.tile([P, D], FP, tag="w_bc")
                row = w_scr[b : b + 1, :]
                w_row_bcast = bass.AP(
                    tensor=row.tensor,
                    offset=row.offset,
                    ap=[[0, P], [1, D]],
                )
                nc.gpsimd.dma_start(out=w_bc, in_=w_row_bcast)

                for j in range(NSUP):
                    ht = hpool.tile([P, SUP, D], FP, tag="ht")
                    nc.sync.dma_start(
                        out=ht,
                        in_=h[b, j * SUP * P : (j + 1) * SUP * P, :].rearrange(
                            "(jt p) d -> p jt d", p=P
                        ),
                    )
                    for jt in range(SUP):
                        ti = j * SUP + jt
                        col = b * TI + ti
                        jd = junk.tile([P, D], FP, tag="jd")
                        # fused (h * w) multiply + free-dim sum on DVE.
                        # (tensor_tensor_reduce crashes the exec unit on this
                        # runtime; scalar_tensor_tensor's accum path works.)
                        nc.vector.scalar_tensor_tensor(
                            out=jd,
                            in0=ht[:, jt, :],
                            scalar=1.0,
                            in1=w_bc,
                            op0=ALU.mult,
                            op1=ALU.mult,
                            accum_out=e_all[:, col : col + 1],
                        )
                        if b == 0 or ti in DVE_HSUM_TI:
                            nc.vector.tensor_reduce(
                                out=hs_all[:, col : col + 1],
                                in_=ht[:, jt, :],
                                axis=mybir.AxisListType.X,
                                op=ALU.add,
                            )
                        else:
                            ja = junk.tile([P, D], FP, tag="ja")
                            nc.scalar.activation(
                                out=ja,
                                in_=ht[:, jt, :],
                                func=AF.Copy,
                                accum_out=hs_all[:, col : col + 1],
                            )

                # ---- stage 2 for batch b: softmax over its 16 columns ----
                if variant == "s1":
                    continue
                cols = slice(b * TI, (b + 1) * TI)

                colmax = small.tile([P, 1], FP, tag="colmax")
                nc.vector.tensor_reduce(
                    out=colmax, in_=e_all[:, cols], axis=mybir.AxisListType.X,
                    op=ALU.max,
                )
                cm_ps = psum2.tile([1, P], FP, tag="cm_ps")
                nc.tensor.transpose(cm_ps, in_=colmax, identity=ident)
                bmax = small.tile([1, 1], FP, tag="bmax")
                nc.vector.tensor_reduce(
                    out=bmax, in_=cm_ps, axis=mybir.AxisListType.X, op=ALU.max
                )
                # -bmax broadcast down the partitions: (-1s)^T @ bmax
                nb_ps = psum2.tile([P, 1], FP, tag="nb_ps")
                nc.tensor.matmul(nb_ps, lhsT=neg_1x128, rhs=bmax)
                nbmax = small.tile([P, 1], FP, tag="nbmax")
                nc.vector.tensor_copy(out=nbmax, in_=nb_ps)

                pscol = small.tile([P, 1], FP, tag="pscol")
                nc.scalar.activation(
                    out=exp_all[:, cols],
                    in_=e_all[:, cols],
                    func=AF.Exp,
                    bias=nbmax,
                    scale=1.0,
                    accum_out=pscol,
                )
                # total = sum_p pscol[p] via ones matmul
                es_ps = psum2.tile([1, 1], FP, tag="es_ps")
                nc.tensor.matmul(es_ps, lhsT=pscol, rhs=ones_128x1)
                rcp = small.tile([1, 1], FP, tag="rcp")
                nc.vector.reciprocal(out=rcp, in_=es_ps)
                rc_ps = psum2.tile([P, 1], FP, tag="rc_ps")
                nc.tensor.matmul(rc_ps, lhsT=ones_1x128, rhs=rcp)
                rcp_bc = small.tile([P, 1], FP, tag="rcp_bc")
                nc.vector.tensor_copy(out=rcp_bc, in_=rc_ps)

                cbuf = small.tile([P, TI], FP, tag="cbuf")
                nc.vector.tensor_tensor(
                    out=cbuf, in0=exp_all[:, cols], in1=hs_all[:, cols],
                    op=ALU.mult,
                )
                nc.vector.tensor_scalar_mul(out=cbuf, in0=cbuf, scalar1=rcp_bc)

                ct_ps = psum2.tile([TI, P], FP, tag="ct_ps")
                nc.tensor.transpose(ct_ps, in_=cbuf, identity=ident)
                ct_sb = small.tile([TI, P], FP, tag="ct_sb")
                nc.scalar.copy(out=ct_sb, in_=ct_ps)
                nc.sync.dma_start(
                    out=c_out[b, :].rearrange("(i p) -> i p", p=P), in_=ct_sb
                )

            if variant == "s1":
                nc.sync.dma_start(out=c_out[:, :P], in_=e_all)
                nc.sync.dma_start(out=c_out[:, P:], in_=hs_all)


_CACHE = {}


def _build():
    if "nc" not in _CACHE:
        nc = bacc.Bacc(
            "TRN2", target_bir_lowering=False, debug=False, num_devices=N_CORES
        )
        with tile.TileContext(nc) as tc:
            _emit(nc, tc)
        nc.compile()
        _CACHE["nc"] = nc
    return _CACHE["nc"]


def kernel(s, h, phi_w, phi_b, psi_w, psi_b=None, **_unused):
    s = np.ascontiguousarray(np.asarray(s, dtype=np.float32))
    h = np.ascontiguousarray(np.asarray(h, dtype=np.float32))
    phi_w = np.ascontiguousarray(np.asarray(phi_w, dtype=np.float32))
    phi_b = np.ascontiguousarray(np.asarray(phi_b, dtype=np.float32))
    psi_w = np.ascontiguousarray(np.asarray(psi_w, dtype=np.float32))

    nc = _build()
    in_maps = [
        {
            "s": s[i * B_LOC : (i + 1) * B_LOC],
            "h": h[i * B_LOC : (i + 1) * B_LOC],
            "phi_w": phi_w,
            "phi_b": phi_b,
            "psi_w": psi_w,
        }
        for i in range(N_CORES)
    ]
    res = bass_utils.run_bass_kernel_spmd(nc, in_maps, core_ids=list(range(N_CORES)))
    return np.concatenate(
        [res.results[i]["c"] for i in range(N_CORES)], axis=0
    ).astype(np.float32)
